# revision 13
# baseline (speedup 1.0000x reference)
# Trainium2 Bass kernel for nn_CustomImageCosineSimLoss (N=4096, D=512, 8 cores).
#
# Strategy (sharding_hint): shard image rows across the 8 cores (data parallel
# over i); text features / instruction ids replicated. Each core computes its
# [512, 4096] block of both pairwise matrices and 16 relu partial sums plus
# per-row min-max stats; the host combines the partials with two closed-form
# corrections and divides by N^2 (the "all-reduce").
#
# Math per core (L=512 local rows):
#   device part = sum_ij relu(cos_ij - w_ij)        (over ALL pairs)
# with sim'_ij = that_i . t_j  (= sim_ij / n_i, so the min-max weights
# w_ij = (sim'_ij - mn'_i) * invr'_i match the reference up to an O(1e-7)
# epsilon shift) and cos_ij = ihat_i . that_j.  The host adds the exact
# aligned-pair term sum_aligned (1 - cos) (fp64 group sums, O(N*D)) and
# subtracts its own estimate of the aligned relu terms the device included,
# using the device-exported stats (mirroring the bf16 rounding of invr').
#
# Two phases so no engine ever waits on the DVE stats chain:
#   phase 1 (sim): PE runs all four sim' i-tile sweeps (fp8 DoubleRow, fp32
#     PSUM); ACT copies each [128,1024] PSUM tile to SBUF (bf16); DVE trails
#     with per-half-row min/max reduces, the scalar chain, and the
#     diag(-invr') build (identity * ninvr, bf16) per i-tile.
#   phase 2 (cos): PE runs the cos sweeps (fp8 DoubleRow) and folds the
#     -sim'*invr' term into each PSUM group via the diag matmul; ACT computes
#     relu(pc + mn'*invr') straight off PSUM (per-partition bias) with
#     per-row sum accumulation.  DVE is idle; all stats are long since done.
# Operands arrive pre-normalized/transposed/flat from the host; DMA triggers
# split across the sync and scalar hardware-DGE queues.
import numpy as np
import ml_dtypes

import concourse.mybir as mybir
import concourse.tile as tile
from concourse import bacc
from concourse.bass import ts

BF16 = mybir.dt.bfloat16
F32 = mybir.dt.float32
FP8 = mybir.dt.float8e4
AF = mybir.ActivationFunctionType
OP = mybir.AluOpType
PM = mybir.MatmulPerfMode
nf8 = ml_dtypes.float8_e4m3
nbf = ml_dtypes.bfloat16

N, D, G, NCORES = 4096, 512, 64, 8
L = N // NCORES            # 512 local rows per core
KT = D // 128              # 4 contraction subtiles
KP = KT // 2               # 2 DoubleRow pairs
IT = L // 128              # 4 local i-tiles
JT = N // 512              # 8 j-blocks
WT = JT // 2               # 4 wide (1024-col) tiles per i-tile
EPS_W = 1e-6

_CACHE = {}


def _build_program():
    nc = bacc.Bacc("TRN2", target_bir_lowering=False, debug=False,
                   enable_asserts=True, num_devices=NCORES)

    d_txtj = [nc.dram_tensor(f"txtj{j}", [128, KT * 512], FP8,
                             kind="ExternalInput").ap() for j in range(JT)]
    d_that_all = nc.dram_tensor("that_all", [128, KT * N], FP8,
                                kind="ExternalInput").ap()
    d_that_loc = nc.dram_tensor("that_loc", [128, KT * L], FP8,
                                kind="ExternalInput").ap()
    d_ihat_loc = nc.dram_tensor("ihat_loc", [128, KT * L], FP8,
                                kind="ExternalInput").ap()
    d_ident = nc.dram_tensor("ident", [128, 128], BF16,
                             kind="ExternalInput").ap()
    d_partials = nc.dram_tensor("partials", [128, IT * WT], F32,
                                kind="ExternalOutput").ap()
    d_stats = nc.dram_tensor("stats_out", [128, 2 * IT], F32,
                             kind="ExternalOutput").ap()

    with tile.TileContext(nc) as tc:
        with (
            tc.tile_pool(name="persist", bufs=1) as pp,
            tc.tile_pool(name="psA", bufs=2, space="PSUM") as ppsA,
            tc.tile_pool(name="psB", bufs=2, space="PSUM") as ppsB,
        ):
            # loads, in need order, split across the two hardware-DGE queues
            that_loc = pp.tile([128, KT * L], FP8)
            nc.sync.dma_start(that_loc[:], d_that_loc)
            that_loc_v = that_loc[:].rearrange("p (c i) -> p c i", c=KT)

            txtj = []
            for j in range(JT):
                t_ = pp.tile([128, KT * 512], FP8, tag=f"txtj{j}")
                (nc.sync if j % 2 == 0 else nc.scalar).dma_start(t_[:], d_txtj[j])
                txtj.append(t_[:].rearrange("p (c j) -> p c j", c=KT))

            ihat_loc = pp.tile([128, KT * L], FP8)
            nc.scalar.dma_start(ihat_loc[:], d_ihat_loc)
            ihat_loc_v = ihat_loc[:].rearrange("p (c i) -> p c i", c=KT)
            ident = pp.tile([128, 128], BF16)
            nc.scalar.dma_start(ident[:], d_ident)

            that_all = pp.tile([128, KT * N], FP8)
            nc.sync.dma_start(that_all[:], d_that_all)
            that_all_v = that_all[:].rearrange("p (c j) -> p c j", c=KT)

            # persistent working state (no rotating SBUF pools)
            sims = pp.tile([128, IT * N], BF16)       # all four sim' rows
            sims_v = sims[:].rearrange("p (t j) -> p t j", t=IT)
            diags = pp.tile([128, IT * 128], BF16)    # diag(-invr') per it
            diags_v = diags[:].rearrange("p (t j) -> p t j", t=IT)
            scs = pp.tile([128, IT * 8], F32)         # minmax scratch per it
            scs_v = scs[:].rearrange("p (t c) -> p t c", t=IT)
            junk = pp.tile([128, 1024], BF16)
            comb = pp.tile([128, IT * WT], F32)
            stats_sb = pp.tile([128, 2 * IT], F32)    # invr / mninvr per it

            # ---------------- phase 1: sim' sweeps ----------------
            for it in range(IT):
                sc = scs_v[:, it, :]
                for w in range(WT):
                    ps = ppsA.tile([128, 1024], F32, tag="mmA")
                    for h in range(2):
                        jt = 2 * w + h
                        for kp in range(KP):
                            nc.tensor.matmul(
                                ps[:, ts(h, 512)],
                                that_loc_v[:, 2 * kp:2 * kp + 2, ts(it, 128)],
                                txtj[jt][:, 2 * kp:2 * kp + 2, :],
                                start=(kp == 0), stop=(kp == KP - 1),
                                perf_mode=PM.DoubleRow)
                    nc.scalar.copy(sims_v[:, it, ts(w, 1024)], ps[:])
                    if w % 2 == 1:
                        hh = w // 2
                        half = sims_v[:, it, ts(hh, 2048)]
                        nc.vector.tensor_reduce(
                            out=sc[:, hh:hh + 1], in_=half,
                            axis=mybir.AxisListType.X, op=OP.min)
                        nc.vector.tensor_reduce(
                            out=sc[:, 2 + hh:3 + hh], in_=half,
                            axis=mybir.AxisListType.X, op=OP.max)
                # stats chain for this i-tile
                nc.vector.tensor_reduce(out=sc[:, 4:5], in_=sc[:, 0:2],
                                        axis=mybir.AxisListType.X, op=OP.min)
                nc.vector.tensor_reduce(out=sc[:, 5:6], in_=sc[:, 2:4],
                                        axis=mybir.AxisListType.X, op=OP.max)
                nc.vector.tensor_tensor(out=sc[:, 6:7], in0=sc[:, 5:6],
                                        in1=sc[:, 4:5], op=OP.subtract)
                nc.vector.tensor_scalar_add(out=sc[:, 6:7], in0=sc[:, 6:7],
                                            scalar1=EPS_W)
                invr = stats_sb[:, 2 * it:2 * it + 1]
                nc.vector.reciprocal(invr, sc[:, 6:7])
                nc.vector.tensor_scalar_mul(out=sc[:, 7:8], in0=invr,
                                            scalar1=-1.0)
                mninvr = stats_sb[:, 2 * it + 1:2 * it + 2]
                nc.vector.tensor_tensor(out=mninvr, in0=sc[:, 4:5], in1=invr,
                                        op=OP.mult)
                nc.vector.tensor_scalar_mul(out=diags_v[:, it, :],
                                            in0=ident[:], scalar1=sc[:, 7:8])

            # ---------------- phase 2: cos sweeps + relu ----------------
            for it in range(IT):
                mninvr = stats_sb[:, 2 * it + 1:2 * it + 2]
                for w in range(WT):
                    pc = ppsB.tile([128, 1024], F32, tag="mmB")
                    for h in range(2):
                        jt = 2 * w + h
                        for kp in range(KP):
                            nc.tensor.matmul(
                                pc[:, ts(h, 512)],
                                ihat_loc_v[:, 2 * kp:2 * kp + 2, ts(it, 128)],
                                that_all_v[:, 2 * kp:2 * kp + 2, ts(jt, 512)],
                                start=(kp == 0), stop=False,
                                perf_mode=PM.DoubleRow)
                        nc.tensor.matmul(pc[:, ts(h, 512)], diags_v[:, it, :],
                                         sims_v[:, it, ts(jt, 512)],
                                         start=False, stop=True)
                    # relu(pc + mn'*invr') with per-row sum accumulation
                    nc.scalar.activation(
                        out=junk[:], in_=pc[:], func=AF.Relu, bias=mninvr,
                        scale=1.0,
                        accum_out=comb[:, it * WT + w:it * WT + w + 1])

            nc.sync.dma_start(d_partials, comb[:])
            nc.sync.dma_start(d_stats, stats_sb[:])

    nc.compile()
    return nc


def _flat_dmajor(arr_T8, cols):
    # [D, cols] d-major -> flat SBUF layout [128, KT*cols]
    return np.ascontiguousarray(
        arr_T8.reshape(KT, 128, cols).transpose(1, 0, 2).reshape(128, KT * cols))


def _host_prep(image_features, text_features, instr_d):
    img = np.asarray(image_features, np.float64)
    txt = np.asarray(text_features, np.float64)
    ins = np.asarray(instr_d).astype(np.int64)

    nt = np.linalg.norm(txt, axis=1)
    ni = np.linalg.norm(img, axis=1)
    that = txt / nt[:, None]
    ihat = img / ni[:, None]

    txt_T8 = np.ascontiguousarray(txt.T.astype(np.float32)).astype(nf8)
    that_T8 = np.ascontiguousarray(that.T.astype(np.float32)).astype(nf8)

    shared = {"that_all": _flat_dmajor(that_T8, N),
              "ident": np.eye(128, dtype=nbf)}
    for j in range(JT):
        shared[f"txtj{j}"] = _flat_dmajor(
            np.ascontiguousarray(txt_T8[:, j * 512:(j + 1) * 512]), 512)

    in_maps = []
    for c in range(NCORES):
        sl = slice(c * L, (c + 1) * L)
        m = dict(shared)
        m["that_loc"] = _flat_dmajor(np.ascontiguousarray(that_T8[:, sl]), L)
        m["ihat_loc"] = _flat_dmajor(
            np.ascontiguousarray(ihat[sl].T.astype(np.float32)).astype(nf8), L)
        in_maps.append(m)

    # exact aligned-pair contribution sum_aligned (1 - cos), fp64 on host
    cnt = np.bincount(ins, minlength=G).astype(np.float64)
    IH = np.zeros((G, D))
    np.add.at(IH, ins, ihat)
    TH = np.zeros((G, D))
    np.add.at(TH, ins, that)
    corr = float((cnt ** 2).sum() - (IH * TH).sum())
    return in_maps, corr, ins, txt, that, ihat


def _aligned_relu_sub(res, ins, txt, that, ihat):
    # Reconstruct per-row invr / mn*invr from the device stats dumps
    # (mirroring the bf16 rounding the diag matmul applied to invr), then
    # estimate the aligned-pair relu terms the device summed (to subtract).
    invr = np.zeros(N)
    mninvr = np.zeros(N)
    for c, r in enumerate(res.results):
        st = np.asarray(r["stats_out"], np.float64)     # [128, 2*IT]
        for it in range(IT):
            rows = slice(c * L + it * 128, c * L + it * 128 + 128)
            invr[rows] = st[:, 2 * it].astype(np.float32).astype(nbf)
            mninvr[rows] = st[:, 2 * it + 1]
    sub = 0.0
    for g in range(G):
        idx = np.where(ins == g)[0]
        if idx.size == 0:
            continue
        cosg = ihat[idx] @ that[idx].T
        simg = that[idx] @ txt[idx].T
        arg = cosg - simg * invr[idx][:, None] + mninvr[idx][:, None]
        sub += np.maximum(arg, 0.0).sum()
    return sub


def kernel(**inputs) -> np.ndarray:
    from concourse.bass_utils import run_bass_kernel_spmd

    if "nc" not in _CACHE:
        _CACHE["nc"] = _build_program()
    nc = _CACHE["nc"]
    in_maps, corr, ins, txt, that, ihat = _host_prep(**inputs)
    res = run_bass_kernel_spmd(nc, in_maps, core_ids=list(range(NCORES)),
                               trace=False)
    _CACHE["last_results"] = res
    total = np.float64(corr)
    for r in res.results:
        total += np.asarray(r["partials"], np.float64).sum()
    total -= _aligned_relu_sub(res, ins, txt, that, ihat)
    return np.float32(total / (N * N))


# revision 14
# speedup vs baseline: 1.0102x; 1.0102x over previous
# Trainium2 Bass kernel for nn_CustomImageCosineSimLoss (N=4096, D=512, 8 cores).
#
# Strategy (sharding_hint): shard image rows across the 8 cores (data parallel
# over i); text features / instruction ids replicated. Each core computes its
# [512, 4096] block of both pairwise matrices and 16 relu partial sums plus
# per-row min-max stats; the host combines the partials with two closed-form
# corrections and divides by N^2 (the "all-reduce").
#
# Math per core (L=512 local rows):
#   device part = sum_ij relu(cos_ij - w_ij)        (over ALL pairs)
# with sim'_ij = that_i . t_j  (= sim_ij / n_i, so the min-max weights
# w_ij = (sim'_ij - mn'_i) * invr'_i match the reference up to an O(1e-7)
# epsilon shift) and cos_ij = ihat_i . that_j.  The host adds the exact
# aligned-pair term sum_aligned (1 - cos) (fp64 group sums, O(N*D)) and
# subtracts its own estimate of the aligned relu terms the device included,
# using the device-exported stats (mirroring the bf16 rounding of invr').
#
# Two phases so no engine ever waits on the DVE stats chain:
#   phase 1 (sim): PE runs all four sim' i-tile sweeps (fp8 DoubleRow, fp32
#     PSUM); ACT copies each [128,1024] PSUM tile to SBUF (bf16); DVE trails
#     with per-half-row min/max reduces, the scalar chain, and the
#     diag(-invr') build (identity * ninvr, bf16) per i-tile.
#   phase 2 (cos): PE runs the cos sweeps (fp8 DoubleRow) and folds the
#     -sim'*invr' term into each PSUM group via the diag matmul; ACT computes
#     relu(pc + mn'*invr') straight off PSUM (per-partition bias) with
#     per-row sum accumulation.  DVE is idle; all stats are long since done.
# Operands arrive pre-normalized/transposed/flat from the host; DMA triggers
# split across the sync and scalar hardware-DGE queues.
import numpy as np
import ml_dtypes

import concourse.mybir as mybir
import concourse.tile as tile
from concourse import bacc
from concourse.bass import ts

BF16 = mybir.dt.bfloat16
F32 = mybir.dt.float32
FP8 = mybir.dt.float8e4
AF = mybir.ActivationFunctionType
OP = mybir.AluOpType
PM = mybir.MatmulPerfMode
nf8 = ml_dtypes.float8_e4m3
nbf = ml_dtypes.bfloat16

N, D, G, NCORES = 4096, 512, 64, 8
L = N // NCORES            # 512 local rows per core
KT = D // 128              # 4 contraction subtiles
KP = KT // 2               # 2 DoubleRow pairs
IT = L // 128              # 4 local i-tiles
JT = N // 512              # 8 j-blocks
WT = JT // 2               # 4 wide (1024-col) tiles per i-tile
EPS_W = 1e-6

_CACHE = {}


def _build_program():
    nc = bacc.Bacc("TRN2", target_bir_lowering=False, debug=False,
                   enable_asserts=True, num_devices=NCORES)

    d_txtj = [nc.dram_tensor(f"txtj{j}", [128, KT * 512], FP8,
                             kind="ExternalInput").ap() for j in range(JT)]
    d_that_all = nc.dram_tensor("that_all", [128, KT * N], FP8,
                                kind="ExternalInput").ap()
    d_that_loc = nc.dram_tensor("that_loc", [128, KT * L], FP8,
                                kind="ExternalInput").ap()
    d_ihat_loc = nc.dram_tensor("ihat_loc", [128, KT * L], FP8,
                                kind="ExternalInput").ap()
    d_ident = nc.dram_tensor("ident", [128, 128], BF16,
                             kind="ExternalInput").ap()
    d_partials = nc.dram_tensor("partials", [128, IT * WT], F32,
                                kind="ExternalOutput").ap()
    d_stats = nc.dram_tensor("stats_out", [128, 2 * IT], F32,
                             kind="ExternalOutput").ap()

    with tile.TileContext(nc) as tc:
        with (
            tc.tile_pool(name="persist", bufs=1) as pp,
            tc.tile_pool(name="psA", bufs=2, space="PSUM") as ppsA,
            tc.tile_pool(name="psB", bufs=2, space="PSUM") as ppsB,
        ):
            # loads, in need order, split across the two hardware-DGE queues
            that_loc = pp.tile([128, KT * L], FP8)
            nc.sync.dma_start(that_loc[:], d_that_loc)
            that_loc_v = that_loc[:].rearrange("p (c i) -> p c i", c=KT)

            txtj = []
            for j in range(JT):
                t_ = pp.tile([128, KT * 512], FP8, tag=f"txtj{j}")
                (nc.sync if j % 2 == 0 else nc.scalar).dma_start(t_[:], d_txtj[j])
                txtj.append(t_[:].rearrange("p (c j) -> p c j", c=KT))

            ihat_loc = pp.tile([128, KT * L], FP8)
            nc.scalar.dma_start(ihat_loc[:], d_ihat_loc)
            ihat_loc_v = ihat_loc[:].rearrange("p (c i) -> p c i", c=KT)
            ident = pp.tile([128, 128], BF16)
            nc.scalar.dma_start(ident[:], d_ident)

            that_all = pp.tile([128, KT * N], FP8)
            nc.sync.dma_start(that_all[:], d_that_all)
            that_all_v = that_all[:].rearrange("p (c j) -> p c j", c=KT)

            # persistent working state (no rotating SBUF pools)
            sims = pp.tile([128, IT * N], BF16)       # all four sim' rows
            sims_v = sims[:].rearrange("p (t j) -> p t j", t=IT)
            diags = pp.tile([128, IT * 128], BF16)    # diag(-invr') per it
            diags_v = diags[:].rearrange("p (t j) -> p t j", t=IT)
            scs = pp.tile([128, IT * 8], F32)         # minmax scratch per it
            scs_v = scs[:].rearrange("p (t c) -> p t c", t=IT)
            junk = pp.tile([128, 1024], BF16)
            comb = pp.tile([128, IT * WT], F32)
            stats_sb = pp.tile([128, 2 * IT], F32)    # invr / mninvr per it

            # ---------------- phase 1: sim' sweeps ----------------
            for it in range(IT):
                sc = scs_v[:, it, :]
                for w in range(WT):
                    ps = ppsA.tile([128, 1024], F32, tag="mmA")
                    for h in range(2):
                        jt = 2 * w + h
                        for kp in range(KP):
                            nc.tensor.matmul(
                                ps[:, ts(h, 512)],
                                that_loc_v[:, 2 * kp:2 * kp + 2, ts(it, 128)],
                                txtj[jt][:, 2 * kp:2 * kp + 2, :],
                                start=(kp == 0), stop=(kp == KP - 1),
                                perf_mode=PM.DoubleRow)
                    nc.scalar.copy(sims_v[:, it, ts(w, 1024)], ps[:])
                    if w % 2 == 1:
                        hh = w // 2
                        half = sims_v[:, it, ts(hh, 2048)]
                        nc.vector.tensor_reduce(
                            out=sc[:, hh:hh + 1], in_=half,
                            axis=mybir.AxisListType.X, op=OP.min)
                        nc.vector.tensor_reduce(
                            out=sc[:, 2 + hh:3 + hh], in_=half,
                            axis=mybir.AxisListType.X, op=OP.max)
                # stats chain for this i-tile; high priority so the tile
                # scheduler doesn't queue later i-tiles' big reduces first
                # (diag gates the whole cos phase)
                with tc.high_priority():
                    nc.vector.tensor_reduce(out=sc[:, 4:5], in_=sc[:, 0:2],
                                            axis=mybir.AxisListType.X, op=OP.min)
                    nc.vector.tensor_reduce(out=sc[:, 5:6], in_=sc[:, 2:4],
                                            axis=mybir.AxisListType.X, op=OP.max)
                    nc.vector.tensor_tensor(out=sc[:, 6:7], in0=sc[:, 5:6],
                                            in1=sc[:, 4:5], op=OP.subtract)
                    nc.vector.tensor_scalar_add(out=sc[:, 6:7], in0=sc[:, 6:7],
                                                scalar1=EPS_W)
                    invr = stats_sb[:, 2 * it:2 * it + 1]
                    nc.vector.reciprocal(invr, sc[:, 6:7])
                    nc.vector.tensor_scalar_mul(out=sc[:, 7:8], in0=invr,
                                                scalar1=-1.0)
                    mninvr = stats_sb[:, 2 * it + 1:2 * it + 2]
                    nc.vector.tensor_tensor(out=mninvr, in0=sc[:, 4:5],
                                            in1=invr, op=OP.mult)
                    nc.vector.tensor_scalar_mul(out=diags_v[:, it, :],
                                                in0=ident[:], scalar1=sc[:, 7:8])

            # ---------------- phase 2: cos sweeps + relu ----------------
            for it in range(IT):
                mninvr = stats_sb[:, 2 * it + 1:2 * it + 2]
                for w in range(WT):
                    pc = ppsB.tile([128, 1024], F32, tag="mmB")
                    for h in range(2):
                        jt = 2 * w + h
                        for kp in range(KP):
                            nc.tensor.matmul(
                                pc[:, ts(h, 512)],
                                ihat_loc_v[:, 2 * kp:2 * kp + 2, ts(it, 128)],
                                that_all_v[:, 2 * kp:2 * kp + 2, ts(jt, 512)],
                                start=(kp == 0), stop=False,
                                perf_mode=PM.DoubleRow)
                        nc.tensor.matmul(pc[:, ts(h, 512)], diags_v[:, it, :],
                                         sims_v[:, it, ts(jt, 512)],
                                         start=False, stop=True)
                    # relu(pc + mn'*invr') with per-row sum accumulation
                    nc.scalar.activation(
                        out=junk[:], in_=pc[:], func=AF.Relu, bias=mninvr,
                        scale=1.0,
                        accum_out=comb[:, it * WT + w:it * WT + w + 1])

            nc.sync.dma_start(d_partials, comb[:])
            nc.sync.dma_start(d_stats, stats_sb[:])

    nc.compile()
    return nc


def _flat_dmajor(arr_T8, cols):
    # [D, cols] d-major -> flat SBUF layout [128, KT*cols]
    return np.ascontiguousarray(
        arr_T8.reshape(KT, 128, cols).transpose(1, 0, 2).reshape(128, KT * cols))


def _host_prep(image_features, text_features, instr_d):
    img = np.asarray(image_features, np.float64)
    txt = np.asarray(text_features, np.float64)
    ins = np.asarray(instr_d).astype(np.int64)

    nt = np.linalg.norm(txt, axis=1)
    ni = np.linalg.norm(img, axis=1)
    that = txt / nt[:, None]
    ihat = img / ni[:, None]

    txt_T8 = np.ascontiguousarray(txt.T.astype(np.float32)).astype(nf8)
    that_T8 = np.ascontiguousarray(that.T.astype(np.float32)).astype(nf8)

    shared = {"that_all": _flat_dmajor(that_T8, N),
              "ident": np.eye(128, dtype=nbf)}
    for j in range(JT):
        shared[f"txtj{j}"] = _flat_dmajor(
            np.ascontiguousarray(txt_T8[:, j * 512:(j + 1) * 512]), 512)

    in_maps = []
    for c in range(NCORES):
        sl = slice(c * L, (c + 1) * L)
        m = dict(shared)
        m["that_loc"] = _flat_dmajor(np.ascontiguousarray(that_T8[:, sl]), L)
        m["ihat_loc"] = _flat_dmajor(
            np.ascontiguousarray(ihat[sl].T.astype(np.float32)).astype(nf8), L)
        in_maps.append(m)

    # exact aligned-pair contribution sum_aligned (1 - cos), fp64 on host
    cnt = np.bincount(ins, minlength=G).astype(np.float64)
    IH = np.zeros((G, D))
    np.add.at(IH, ins, ihat)
    TH = np.zeros((G, D))
    np.add.at(TH, ins, that)
    corr = float((cnt ** 2).sum() - (IH * TH).sum())
    return in_maps, corr, ins, txt, that, ihat


def _aligned_relu_sub(res, ins, txt, that, ihat):
    # Reconstruct per-row invr / mn*invr from the device stats dumps
    # (mirroring the bf16 rounding the diag matmul applied to invr), then
    # estimate the aligned-pair relu terms the device summed (to subtract).
    invr = np.zeros(N)
    mninvr = np.zeros(N)
    for c, r in enumerate(res.results):
        st = np.asarray(r["stats_out"], np.float64)     # [128, 2*IT]
        for it in range(IT):
            rows = slice(c * L + it * 128, c * L + it * 128 + 128)
            invr[rows] = st[:, 2 * it].astype(np.float32).astype(nbf)
            mninvr[rows] = st[:, 2 * it + 1]
    sub = 0.0
    for g in range(G):
        idx = np.where(ins == g)[0]
        if idx.size == 0:
            continue
        cosg = ihat[idx] @ that[idx].T
        simg = that[idx] @ txt[idx].T
        arg = cosg - simg * invr[idx][:, None] + mninvr[idx][:, None]
        sub += np.maximum(arg, 0.0).sum()
    return sub


def kernel(**inputs) -> np.ndarray:
    from concourse.bass_utils import run_bass_kernel_spmd

    if "nc" not in _CACHE:
        _CACHE["nc"] = _build_program()
    nc = _CACHE["nc"]
    in_maps, corr, ins, txt, that, ihat = _host_prep(**inputs)
    res = run_bass_kernel_spmd(nc, in_maps, core_ids=list(range(NCORES)),
                               trace=False)
    _CACHE["last_results"] = res
    total = np.float64(corr)
    for r in res.results:
        total += np.asarray(r["partials"], np.float64).sum()
    total -= _aligned_relu_sub(res, ins, txt, that, ihat)
    return np.float32(total / (N * N))
